# revision 1
# baseline (speedup 1.0000x reference)
"""Trainium2 Bass kernel for nn_HadamardExpansionV2 (topk_masking).

Sharding: data-parallel over batch B=16 across 8 cores (2 samples/core);
weights replicated. CrossHadaNorm batch stats via AllReduce of per-channel
sum/sumsq.

Per-core pipeline (channels-on-partitions layout):
  1. conv1x1: y[o,hw] = fc_w @ x + BN affine (PE matmul, per-partition
     scale/bias epilogue on DVE), streamed to DRAM output rows 0..511.
  2. logits path (exact fp32, independent of conv dtype): xbar = mean_hw(x),
     y_mean = W @ xbar (+affine), logits = z.T @ eva_w.T + eva_b -> [2, 512].
  3. top-32 via vector.max / max_index / match_replace (4 rounds of 8).
  4. gather 32 selected rows of y from DRAM via indirect_dma_start.
  5. hadamard expansion: A = G_hi @ x_sel, B = G_hj @ x_sel (PE, 0/1 consts),
     prod = A * B on DVE; per-channel sum (DVE reduce) + sumsq (ACT Square
     accum_out).
  6. AllReduce(add) of [sum|sumsq] across 8 cores; mean/var/rstd; final
     per-partition affine; stream prod rows 512..1007 to DRAM.
"""

import os
import sys

import numpy as np

for _p in ("/opt/trn_rl_repo", os.path.expanduser("~/.axon_site/_ro/trn_rl_repo")):
    if os.path.isdir(_p) and _p not in sys.path:
        sys.path.insert(0, _p)

import concourse.bass as bass
import concourse.mybir as mybir
import concourse.tile as tile
from concourse import bacc
from concourse.bass_utils import run_bass_kernel_spmd

C1 = 512
CS = 32
CSE = 496  # 32*31/2
HWD = 1024  # H*W
B = 16
NCORES = 8
SPC = B // NCORES  # samples per core
P = 128
KC = C1 // P  # 4 contraction chunks
MC = C1 // P  # 4 output-channel chunks
NF = 512  # matmul free dim (PSUM bank)
NNC = HWD // NF  # 2 free chunks
EPS = 1e-5
NTOT = float(B * HWD)

HI, HJ = np.triu_indices(CS, k=1)

F32 = mybir.dt.float32
F32R = mybir.dt.float32r
U32 = mybir.dt.uint32

# M-chunking of the 496 expanded channels: 128,128,128,112
EXP_M = [(0, 128), (128, 128), (256, 128), (384, 112)]

# conv matmul dtype: "f32" (exact, 4 cyc/row) or "f32r" (fast, reduced precision)
CONV_MODE = os.environ.get("CONV_MODE", "f32")
# gather (0/1 selection matrix) matmul dtype
GATHER_MODE = os.environ.get("GATHER_MODE", "f32")


def _cast(ap, mode):
    return ap.bitcast(F32R) if mode == "f32r" else ap


def build_program():
    nc = bacc.Bacc(
        "TRN2",
        target_bir_lowering=False,
        debug=False,
        num_devices=NCORES,
    )

    # ---------------- I/O ----------------
    xs = nc.dram_tensor("xs", [SPC, C1, HWD], F32, kind="ExternalInput")
    fc_wT = nc.dram_tensor("fc_wT", [C1, C1], F32, kind="ExternalInput")  # [c, o]
    eva_wT = nc.dram_tensor("eva_wT", [C1, C1], F32, kind="ExternalInput")  # [c, o']
    fc_b = nc.dram_tensor("fc_b", [C1], F32, kind="ExternalInput")
    bn_gamma = nc.dram_tensor("bn_gamma", [C1], F32, kind="ExternalInput")
    bn_beta = nc.dram_tensor("bn_beta", [C1], F32, kind="ExternalInput")
    bn_mean = nc.dram_tensor("bn_mean", [C1], F32, kind="ExternalInput")
    bn_var = nc.dram_tensor("bn_var", [C1], F32, kind="ExternalInput")
    eva_b = nc.dram_tensor("eva_b", [C1], F32, kind="ExternalInput")
    g_hi = nc.dram_tensor("g_hi", [P, CSE], F32, kind="ExternalInput")  # zero-padded
    g_hj = nc.dram_tensor("g_hj", [P, CSE], F32, kind="ExternalInput")

    outs = [
        nc.dram_tensor(f"out{s}", [C1 + CSE, HWD], F32, kind="ExternalOutput")
        for s in range(SPC)
    ]

    with tile.TileContext(nc) as tc:
        with (
            tc.tile_pool(name="const", bufs=1) as cpool,
            tc.tile_pool(name="xp", bufs=2) as xpool,
            tc.tile_pool(name="yp", bufs=2) as ypool,
            tc.tile_pool(name="prodp", bufs=2) as prodpool,
            tc.tile_pool(name="small", bufs=3) as spool,
            tc.tile_pool(name="acopy", bufs=3) as apool,
            tc.tile_pool(name="psc", bufs=2, space="PSUM") as psum_conv,
            tc.tile_pool(name="psh", bufs=4, space="PSUM") as psum_had,
            tc.tile_pool(name="psm", bufs=2, space="PSUM") as psum_misc,
            tc.tile_pool(name="dram", bufs=1, space="DRAM") as dpool,
        ):
            # ------------ one-time constants ------------
            wT_sb = cpool.tile([P, KC, C1], F32)
            nc.sync.dma_start(wT_sb[:], fc_wT.ap().rearrange("(ko p) o -> p ko o", p=P))
            evaT_sb = cpool.tile([P, KC, C1], F32)
            nc.sync.dma_start(
                evaT_sb[:], eva_wT.ap().rearrange("(ko p) o -> p ko o", p=P)
            )
            ghi_sb = cpool.tile([P, CSE], F32)
            nc.sync.dma_start(ghi_sb[:], g_hi.ap())
            ghj_sb = cpool.tile([P, CSE], F32)
            nc.sync.dma_start(ghj_sb[:], g_hj.ap())

            # per-channel vectors as [P, MC] (col m = channels m*128..m*128+127)
            def load_vec(t, nm):
                v = cpool.tile([P, MC], F32, tag=f"v_{nm}", name=f"v_{nm}")
                nc.sync.dma_start(v[:], t.ap().rearrange("(m p) -> p m", p=P))
                return v

            fcb_v = load_vec(fc_b, "fcb")
            gam_v = load_vec(bn_gamma, "gam")
            bet_v = load_vec(bn_beta, "bet")
            mu_v = load_vec(bn_mean, "mu")
            var_v = load_vec(bn_var, "var")

            eps_col = cpool.tile([P, 1], F32)
            nc.vector.memset(eps_col[:], EPS)

            # s = gamma / sqrt(var + eps); b2 = (fc_b - mu) * s + beta
            sd_v = cpool.tile([P, MC], F32)
            nc.scalar.activation(
                sd_v[:], var_v[:], mybir.ActivationFunctionType.Sqrt,
                bias=eps_col[:, 0:1],
            )
            rsd_v = cpool.tile([P, MC], F32)
            nc.vector.reciprocal(rsd_v[:], sd_v[:])
            s_v = cpool.tile([P, MC], F32)
            nc.vector.tensor_mul(s_v[:], gam_v[:], rsd_v[:])
            b2_v = cpool.tile([P, MC], F32)
            nc.vector.tensor_sub(b2_v[:], fcb_v[:], mu_v[:])
            nc.vector.tensor_mul(b2_v[:], b2_v[:], s_v[:])
            nc.vector.tensor_add(b2_v[:], b2_v[:], bet_v[:])

            # eva_b as a [SPC, 512] row tile (replicated rows)
            evab_row = cpool.tile([SPC, C1], F32)
            for s in range(SPC):
                nc.sync.dma_start(
                    evab_row[s : s + 1, :],
                    eva_b.ap().rearrange("(one o) -> one o", one=1),
                )

            # gamma[:32], beta[:32] -> [128, 2] (zero-padded partitions 32+)
            gb32 = cpool.tile([P, 2], F32)
            nc.vector.memset(gb32[:], 0.0)
            nc.sync.dma_start(
                gb32[:CS, 0:1], bn_gamma.ap()[0:CS].rearrange("(p one) -> p one", one=1)
            )
            nc.sync.dma_start(
                gb32[:CS, 1:2], bn_beta.ap()[0:CS].rearrange("(p one) -> p one", one=1)
            )
            # pair affine consts: gpair = G_hi@gamma32 * G_hj@gamma32, bpair likewise
            gpair = cpool.tile([P, MC], F32)  # [p, m] channel m*128+p
            bpair = cpool.tile([P, MC], F32)
            nc.vector.memset(gpair[:], 0.0)
            nc.vector.memset(bpair[:], 0.0)
            for m, (mo, mw) in enumerate(EXP_M):
                pa = psum_misc.tile([P, NF], F32, tag="misc")
                pb = psum_misc.tile([P, NF], F32, tag="misc")
                nc.tensor.matmul(
                    pa[:mw, :2],
                    lhsT=_cast(ghi_sb[:, mo : mo + mw], GATHER_MODE),
                    rhs=_cast(gb32[:], GATHER_MODE),
                    start=True,
                    stop=True,
                )
                nc.tensor.matmul(
                    pb[:mw, :2],
                    lhsT=_cast(ghj_sb[:, mo : mo + mw], GATHER_MODE),
                    rhs=_cast(gb32[:], GATHER_MODE),
                    start=True,
                    stop=True,
                )
                tmp2 = spool.tile([P, 2], F32, tag="tmp2")
                tmp2a = spool.tile([P, 2], F32, tag="tmp2a")
                nc.vector.tensor_copy(tmp2a[:mw, :], pa[:mw, :2])
                nc.vector.tensor_mul(tmp2[:mw, :], tmp2a[:mw, :], pb[:mw, :2])
                nc.vector.tensor_copy(gpair[:mw, m : m + 1], tmp2[:mw, 0:1])
                nc.vector.tensor_copy(bpair[:mw, m : m + 1], tmp2[:mw, 1:2])

            # ------------ per-sample: load x, conv, y->DRAM ------------
            x_sb = [None] * SPC
            y_sb = [None] * SPC
            xbar2 = cpool.tile([P, KC, SPC], F32)  # rhs for y_mean matmul
            for s in range(SPC):
                x_sb[s] = xpool.tile([P, KC, HWD], F32, tag="x", name=f"x{s}")
                nc.sync.dma_start(
                    x_sb[s][:], xs.ap()[s].rearrange("(ko p) n -> p ko n", p=P)
                )
                # xbar[c] = mean_hw(x) / 1024 (for exact logits path)
                xb = spool.tile([P, KC], F32, tag="xb")
                nc.vector.tensor_reduce(
                    xb[:], x_sb[s][:], axis=mybir.AxisListType.X, op=mybir.AluOpType.add
                )
                nc.vector.tensor_scalar_mul(xbar2[:, :, s], xb[:], 1.0 / HWD)

                y_sb[s] = ypool.tile([P, MC, HWD], F32, tag="y", name=f"y{s}")
                for m in range(MC):
                    for n in range(NNC):
                        pt = psum_conv.tile([P, NF], F32, tag="cv")
                        for k in range(KC):
                            nc.tensor.matmul(
                                pt[:],
                                lhsT=_cast(
                                    wT_sb[:, k, m * P : (m + 1) * P], CONV_MODE
                                ),
                                rhs=_cast(
                                    x_sb[s][:, k, n * NF : (n + 1) * NF], CONV_MODE
                                ),
                                start=(k == 0),
                                stop=(k == KC - 1),
                            )
                        # y = psum * s + b2  (per-partition affine)
                        nc.vector.tensor_scalar(
                            y_sb[s][:, m, n * NF : (n + 1) * NF],
                            pt[:],
                            s_v[:, m : m + 1],
                            b2_v[:, m : m + 1],
                            op0=mybir.AluOpType.mult,
                            op1=mybir.AluOpType.add,
                        )
                        nc.sync.dma_start(
                            outs[s].ap()[m * P : (m + 1) * P, n * NF : (n + 1) * NF],
                            y_sb[s][:, m, n * NF : (n + 1) * NF],
                        )

            # ------------ logits + topk (both samples) ------------
            # y_mean[o, s] = W' @ xbar (+ affine) ; z in [P, MC, SPC]
            z_sb = cpool.tile([P, MC, SPC], F32)
            for m in range(MC):
                pm = psum_misc.tile([P, NF], F32, tag="misc")
                for k in range(KC):
                    nc.tensor.matmul(
                        pm[:, :SPC],
                        lhsT=wT_sb[:, k, m * P : (m + 1) * P],
                        rhs=xbar2[:, k, :],
                        start=(k == 0),
                        stop=(k == KC - 1),
                    )
                nc.vector.tensor_scalar(
                    z_sb[:, m, :],
                    pm[:, :SPC],
                    s_v[:, m : m + 1],
                    b2_v[:, m : m + 1],
                    op0=mybir.AluOpType.mult,
                    op1=mybir.AluOpType.add,
                )
            # logits[s, o'] = z.T @ eva_wT + eva_b
            pl = psum_misc.tile([P, NF], F32, tag="misc")
            for k in range(MC):
                nc.tensor.matmul(
                    pl[:SPC, :],
                    lhsT=z_sb[:, k, :],
                    rhs=evaT_sb[:, k, :],
                    start=(k == 0),
                    stop=(k == MC - 1),
                )
            logit = spool.tile([SPC, C1], F32, tag="logit")
            nc.vector.tensor_add(logit[:], pl[:SPC, :], evab_row[:])

            # top-32 per sample, descending: 4 rounds of max8
            idx_all = spool.tile([SPC, CS], U32, tag="idx")
            for r in range(4):
                mx8 = spool.tile([SPC, 8], F32, tag="mx8")
                nc.vector.max(out=mx8[:], in_=logit[:])
                nc.vector.max_index(
                    out=idx_all[:, r * 8 : (r + 1) * 8], in_max=mx8[:], in_values=logit[:]
                )
                if r < 3:
                    nc.vector.match_replace(
                        out=logit[:], in_to_replace=mx8[:], in_values=logit[:],
                        imm_value=-1e30,
                    )

            # idx -> per-partition [CS, 1] columns via DRAM roundtrip
            idx_dram = dpool.tile([SPC, CS], U32)
            nc.sync.dma_start(idx_dram[:], idx_all[:])
            idx_col = [None] * SPC
            for s in range(SPC):
                idx_col[s] = spool.tile([CS, 1], U32, tag=f"idxc{s}", name=f"idxc{s}")
                nc.sync.dma_start(
                    idx_col[s][:],
                    idx_dram[s].rearrange("(p one) -> p one", one=1),
                )

            # ------------ gather + hadamard + stats per sample ------------
            s1p = cpool.tile([P, MC, NNC * SPC], F32)  # per-chunk sums
            s2p = cpool.tile([P, MC, NNC * SPC], F32)
            nc.vector.memset(s1p[:], 0.0)
            nc.vector.memset(s2p[:], 0.0)
            prod_sb = [None] * SPC
            for s in range(SPC):
                xsel = spool.tile([P, HWD], F32, tag="xsel")
                for po in range(CS, P, 32):
                    nc.vector.memset(xsel[po : po + 32, :], 0.0)
                nc.gpsimd.indirect_dma_start(
                    out=xsel[:CS, :],
                    out_offset=None,
                    in_=outs[s].ap()[0:C1, :],
                    in_offset=bass.IndirectOffsetOnAxis(ap=idx_col[s][:, :1], axis=0),
                )
                prod_sb[s] = prodpool.tile([P, MC, HWD], F32, tag="prod", name=f"prod{s}")
                for n in range(NNC):
                    for m, (mo, mw) in enumerate(EXP_M):
                        pa = psum_had.tile([P, NF], F32, tag="had")
                        pb = psum_had.tile([P, NF], F32, tag="had")
                        nc.tensor.matmul(
                            pa[:mw, :],
                            lhsT=_cast(ghi_sb[:, mo : mo + mw], GATHER_MODE),
                            rhs=_cast(xsel[:, n * NF : (n + 1) * NF], GATHER_MODE),
                            start=True,
                            stop=True,
                        )
                        nc.tensor.matmul(
                            pb[:mw, :],
                            lhsT=_cast(ghj_sb[:, mo : mo + mw], GATHER_MODE),
                            rhs=_cast(xsel[:, n * NF : (n + 1) * NF], GATHER_MODE),
                            start=True,
                            stop=True,
                        )
                        a_sb = apool.tile([P, NF], F32, tag="ac")
                        nc.scalar.activation(
                            a_sb[:mw, :], pa[:mw, :], mybir.ActivationFunctionType.Copy
                        )
                        pchunk = prod_sb[s][:, m, n * NF : (n + 1) * NF]
                        nc.vector.tensor_tensor(
                            pchunk[:mw], a_sb[:mw, :], pb[:mw, :],
                            op=mybir.AluOpType.mult,
                        )
                        nc.vector.tensor_reduce(
                            s1p[:mw, m, n * SPC + s : n * SPC + s + 1],
                            pchunk[:mw],
                            axis=mybir.AxisListType.X,
                            op=mybir.AluOpType.add,
                        )
                        sqs = apool.tile([P, NF], F32, tag="sq")
                        nc.scalar.activation(
                            sqs[:mw, :],
                            pchunk[:mw],
                            mybir.ActivationFunctionType.Square,
                            accum_out=s2p[:mw, m, n * SPC + s : n * SPC + s + 1],
                        )

            # ------------ cross-core stats AllReduce ------------
            stats = cpool.tile([P, 2 * MC], F32)
            nc.vector.tensor_reduce(
                stats[:, 0:MC], s1p[:], axis=mybir.AxisListType.X,
                op=mybir.AluOpType.add,
            )
            nc.vector.tensor_reduce(
                stats[:, MC : 2 * MC], s2p[:], axis=mybir.AxisListType.X,
                op=mybir.AluOpType.add,
            )
            cc_in = dpool.tile([P, 2 * MC], F32)
            cc_out = dpool.tile([P, 2 * MC], F32)
            nc.sync.dma_start(cc_in[:], stats[:])
            nc.gpsimd.collective_compute(
                "AllReduce",
                mybir.AluOpType.add,
                replica_groups=[list(range(NCORES))],
                ins=[cc_in.opt()],
                outs=[cc_out.opt()],
            )
            gstats = cpool.tile([P, 2 * MC], F32)
            nc.sync.dma_start(gstats[:], cc_out[:])

            # mean/var -> alpha = rstd*gpair ; beta2 = bpair - mean*alpha
            meanc = cpool.tile([P, MC], F32)
            nc.vector.tensor_scalar_mul(meanc[:], gstats[:, 0:MC], 1.0 / NTOT)
            varc = cpool.tile([P, MC], F32)
            nc.vector.tensor_scalar_mul(varc[:], gstats[:, MC : 2 * MC], 1.0 / NTOT)
            msq = spool.tile([P, MC], F32, tag="msq")
            nc.vector.tensor_mul(msq[:], meanc[:], meanc[:])
            nc.vector.tensor_sub(varc[:], varc[:], msq[:])
            nc.scalar.activation(
                varc[:], varc[:], mybir.ActivationFunctionType.Sqrt,
                bias=eps_col[:, 0:1],
            )
            rstd = cpool.tile([P, MC], F32)
            nc.vector.reciprocal(rstd[:], varc[:])
            alpha = cpool.tile([P, MC], F32)
            nc.vector.tensor_mul(alpha[:], rstd[:], gpair[:])
            beta2 = cpool.tile([P, MC], F32)
            nc.vector.tensor_mul(beta2[:], meanc[:], alpha[:])
            nc.vector.tensor_sub(beta2[:], bpair[:], beta2[:])

            # ------------ normalize + write prod rows ------------
            for s in range(SPC):
                for m, (mo, mw) in enumerate(EXP_M):
                    for n in range(NNC):
                        pchunk = prod_sb[s][:, m, n * NF : (n + 1) * NF]
                        nc.vector.tensor_scalar(
                            pchunk[:mw],
                            pchunk[:mw],
                            alpha[:mw, m : m + 1],
                            beta2[:mw, m : m + 1],
                            op0=mybir.AluOpType.mult,
                            op1=mybir.AluOpType.add,
                        )
                        nc.sync.dma_start(
                            outs[s].ap()[
                                C1 + mo : C1 + mo + mw, n * NF : (n + 1) * NF
                            ],
                            pchunk[:mw],
                        )

    nc.compile()
    return nc


_NC_CACHE = {}


def _get_program():
    key = (CONV_MODE, GATHER_MODE)
    if key not in _NC_CACHE:
        _NC_CACHE[key] = build_program()
    return _NC_CACHE[key]


def _make_consts():
    ghi = np.zeros((P, CSE), np.float32)
    ghj = np.zeros((P, CSE), np.float32)
    ghi[HI, np.arange(CSE)] = 1.0
    ghj[HJ, np.arange(CSE)] = 1.0
    return ghi, ghj


def kernel(x, fc_w, fc_b, bn_gamma, bn_beta, bn_mean, bn_var, eva_w, eva_b):
    x = np.asarray(x, np.float32).reshape(B, C1, HWD)
    fc_wT = np.ascontiguousarray(np.asarray(fc_w, np.float32).T)
    eva_wT = np.ascontiguousarray(np.asarray(eva_w, np.float32).T)
    ghi, ghj = _make_consts()
    shared = dict(
        fc_wT=fc_wT,
        eva_wT=eva_wT,
        fc_b=np.asarray(fc_b, np.float32),
        bn_gamma=np.asarray(bn_gamma, np.float32),
        bn_beta=np.asarray(bn_beta, np.float32),
        bn_mean=np.asarray(bn_mean, np.float32),
        bn_var=np.asarray(bn_var, np.float32),
        eva_b=np.asarray(eva_b, np.float32),
        g_hi=ghi,
        g_hj=ghj,
    )
    in_maps = [
        dict(shared, xs=np.ascontiguousarray(x[i * SPC : (i + 1) * SPC]))
        for i in range(NCORES)
    ]
    nc = _get_program()
    res = run_bass_kernel_spmd(nc, in_maps, list(range(NCORES))).results
    out = np.empty((B, C1 + CSE, HWD), np.float32)
    for i in range(NCORES):
        for s in range(SPC):
            out[i * SPC + s] = res[i][f"out{s}"]
    return out.reshape(B, C1 + CSE, 32, 32)



# revision 7
# speedup vs baseline: 1.7817x; 1.7817x over previous
"""Trainium2 Bass kernel for nn_HadamardExpansionV2 (topk_masking).

Sharding: data-parallel over batch B=16 across 8 cores (2 samples/core);
weights replicated. CrossHadaNorm batch stats via AllReduce of per-channel
sum/sumsq.

Host precompute folds BN into the conv: W2 = diag(s) @ fc_w, b2 = (fc_b -
bn_mean)*s + bn_beta with s = gamma/sqrt(var+eps), so y = W2 @ x + b2.
Logits fold: E = (eva_w @ W2)/1024, f = eva_w @ b2 + eva_b, so
logits = E @ sum_hw(x) + f (computed exact fp32 on device: min top-32
boundary gap is ~1e-4, so the logits path cannot use f32r).

Per-core pipeline:
  1. conv m0/m1 k-waves start as x chunks arrive (f32r matmuls, 1 cyc/row).
  2. logits: Pool-engine row-sums of x -> xbar; 4 exact-f32 matmuls
     (lhsT = xbar [c,2], rhs = E^T chunks) -> [2,512]; DVE adds f.
  3. top-32 via 4 rounds of max8/max_index/match_replace; idx to
     per-partition columns via small DRAM roundtrip.
  4. x_sel computed directly from gathered W2 rows (indirect DMA) +
     PE transpose -> selconv (does NOT wait on the full conv).
  5. hadamard: pa = Ghi@xsel, pb = Ghj@xsel (f32r, 32-part contraction);
     Pool copies pa->SBUF; DVE tensor_tensor_reduce gives prod and s1;
     ACT Square+accum gives s2.
  6. AllReduce(add) of [s1|s2] overlaps the conv tail; then per-channel
     affine (DVE) and streamed prod writes.
"""

import os
import sys

import numpy as np

for _p in ("/opt/trn_rl_repo", os.path.expanduser("~/.axon_site/_ro/trn_rl_repo")):
    if os.path.isdir(_p) and _p not in sys.path:
        sys.path.insert(0, _p)

import concourse.bass as bass
import concourse.mybir as mybir
import concourse.tile as tile
from concourse import bacc
from concourse.bass_utils import run_bass_kernel_spmd

C1 = 512
CS = 32
CSE = 496  # 32*31/2
HWD = 1024  # H*W
B = 16
NCORES = 8
SPC = B // NCORES  # samples per core
P = 128
KC = C1 // P  # 4 contraction chunks
MC = C1 // P  # 4 output-channel chunks
NF = 512  # matmul free dim (PSUM bank)
NNC = HWD // NF  # 2 free chunks
EPS = 1e-5
NTOT = float(B * HWD)

HI, HJ = np.triu_indices(CS, k=1)

F32 = mybir.dt.float32
F32R = mybir.dt.float32r
U32 = mybir.dt.uint32

# M-chunking of the 496 expanded channels: 128,128,128,112
EXP_M = [(0, 128), (128, 128), (256, 128), (384, 112)]


def build_program():
    nc = bacc.Bacc(
        "TRN2",
        target_bir_lowering=False,
        debug=False,
        num_devices=NCORES,
    )

    # ---------------- I/O ----------------
    xs = nc.dram_tensor("xs", [SPC, C1, HWD], F32R, kind="ExternalInput")
    w2T = nc.dram_tensor("w2T", [C1, C1], F32R, kind="ExternalInput")  # [c, o]
    w2rows = nc.dram_tensor("w2rows", [C1, C1], F32, kind="ExternalInput")  # [o, c]
    eT = nc.dram_tensor("eT", [C1, C1], F32, kind="ExternalInput")  # [c, o']
    f2 = nc.dram_tensor("f2", [SPC, C1], F32, kind="ExternalInput")
    b2vec = nc.dram_tensor("b2vec", [C1], F32, kind="ExternalInput")
    gpairp = nc.dram_tensor("gpairp", [C1], F32, kind="ExternalInput")  # padded 496->512
    bpairp = nc.dram_tensor("bpairp", [C1], F32, kind="ExternalInput")
    g_hi = nc.dram_tensor("g_hi", [CS, CSE], F32R, kind="ExternalInput")
    g_hj = nc.dram_tensor("g_hj", [CS, CSE], F32R, kind="ExternalInput")
    ident32 = nc.dram_tensor("ident32", [CS, CS], F32, kind="ExternalInput")

    outs = [
        nc.dram_tensor(f"out{s}", [C1 + CSE, HWD], F32, kind="ExternalOutput")
        for s in range(SPC)
    ]

    with tile.TileContext(nc) as tc:
        with (
            tc.tile_pool(name="const", bufs=1) as cpool,
            tc.tile_pool(name="xp", bufs=1) as xpool,
            tc.tile_pool(name="prodp", bufs=1) as prodpool,
            tc.tile_pool(name="yp", bufs=4) as ypool,
            tc.tile_pool(name="acp", bufs=4) as apool,
            tc.tile_pool(name="small", bufs=3) as spool,
            tc.tile_pool(name="psA", bufs=4, space="PSUM") as psA,
            tc.tile_pool(name="psB", bufs=4, space="PSUM") as psB,
            tc.tile_pool(name="dram", bufs=1, space="DRAM") as dpool,
        ):
            # ---- persistent SBUF tiles ----
            wT_sb = cpool.tile([P, KC, C1], F32R, tag="wT", name="wT")
            x_sb = [
                xpool.tile([P, KC, HWD], F32R, tag=f"x{s}", name=f"x{s}")
                for s in range(SPC)
            ]
            eT_sb = cpool.tile([P, KC, C1], F32, tag="eT", name="eT")
            f2_sb = cpool.tile([SPC, C1], F32, tag="f2", name="f2")
            b2v = cpool.tile([P, MC], F32, tag="b2v", name="b2v")
            gpair_v = cpool.tile([P, MC], F32, tag="gpv", name="gpv")
            bpair_v = cpool.tile([P, MC], F32, tag="bpv", name="bpv")
            i32_sb = cpool.tile([CS, CS], F32, tag="i32", name="i32")
            ghi_sb = cpool.tile([CS, CSE], F32R, tag="ghi", name="ghi")
            ghj_sb = cpool.tile([CS, CSE], F32R, tag="ghj", name="ghj")
            xbar2 = cpool.tile([P, KC, SPC], F32, tag="xbar", name="xbar")
            stats = cpool.tile([P, 2 * MC], F32, tag="stats", name="stats")
            s1p = cpool.tile([P, MC, NNC * SPC], F32, tag="s1p", name="s1p")
            s2p = cpool.tile([P, MC, NNC * SPC], F32, tag="s2p", name="s2p")
            gstats = cpool.tile([P, 2 * MC], F32, tag="gstats", name="gstats")
            eps_col = cpool.tile([P, 1], F32, tag="eps", name="eps")
            prod_sb = [
                prodpool.tile([P, MC, HWD], F32, tag=f"prod{s}", name=f"prod{s}")
                for s in range(SPC)
            ]

            # ---- input DMAs: conv weights + x interleaved per k-chunk ----
            for k in range(KC):
                nc.sync.dma_start(
                    wT_sb[:, k, :], w2T.ap()[k * P : (k + 1) * P, :]
                )
                for s in range(SPC):
                    nc.sync.dma_start(
                        x_sb[s][:, k, :], xs.ap()[s][k * P : (k + 1) * P, :]
                    )
            nc.sync.dma_start(eT_sb[:], eT.ap().rearrange("(ko p) o -> p ko o", p=P))
            nc.sync.dma_start(f2_sb[:], f2.ap())
            nc.sync.dma_start(b2v[:], b2vec.ap().rearrange("(m p) -> p m", p=P))
            nc.sync.dma_start(gpair_v[:], gpairp.ap().rearrange("(m p) -> p m", p=P))
            nc.sync.dma_start(bpair_v[:], bpairp.ap().rearrange("(m p) -> p m", p=P))
            nc.sync.dma_start(i32_sb[:], ident32.ap())
            nc.sync.dma_start(ghi_sb[:], g_hi.ap())
            nc.sync.dma_start(ghj_sb[:], g_hj.ap())

            # ---- Pool: memsets + per-chunk x row-sums (for logits) ----
            nc.gpsimd.memset(stats[:], 0.0)
            nc.gpsimd.memset(s1p[:], 0.0)
            nc.gpsimd.memset(s2p[:], 0.0)
            nc.gpsimd.memset(eps_col[:], EPS)
            for k in range(KC):
                for s in range(SPC):
                    nc.vector.tensor_reduce(
                        xbar2[:, k, s : s + 1],
                        x_sb[s].bitcast(F32)[:, k, :],
                        axis=mybir.AxisListType.X,
                        op=mybir.AluOpType.add,
                    )

            # ---- conv helper ----
            def conv_mgroup(m):
                """One output-channel chunk m for both samples: 4 psum banks,
                k-accumulated; epilogue + y write streamed."""
                pts = {}
                for s in range(SPC):
                    for n in range(NNC):
                        pts[(s, n)] = psA.tile([P, NF], F32, tag="cv", name=f"cv{m}_{s}_{n}")
                for k in range(KC):
                    for s in range(SPC):
                        for n in range(NNC):
                            nc.tensor.matmul(
                                pts[(s, n)][:],
                                lhsT=wT_sb[:, k, m * P : (m + 1) * P],
                                rhs=x_sb[s][:, k, n * NF : (n + 1) * NF],
                                start=(k == 0),
                                stop=(k == KC - 1),
                            )
                for s in range(SPC):
                    for n in range(NNC):
                        ych = ypool.tile([P, NF], F32, tag="y")
                        nc.scalar.activation(
                            ych[:],
                            pts[(s, n)][:],
                            mybir.ActivationFunctionType.Identity,
                            bias=b2v[:, m : m + 1],
                            scale=1.0,
                        )
                        nc.sync.dma_start(
                            outs[s].ap()[m * P : (m + 1) * P, n * NF : (n + 1) * NF],
                            ych[:],
                        )

            # conv m0/m1 early (fills PE while x streams in)
            conv_mgroup(0)
            conv_mgroup(1)

            # ---- logits: exact f32, lhsT = xbar chunks, rhs = E^T ----
            pl = psB.tile([P, NF], F32, tag="pa", name="pl")
            for k in range(KC):
                nc.tensor.matmul(
                    pl[:SPC, :],
                    lhsT=xbar2[:, k, :],
                    rhs=eT_sb[:, k, :],
                    start=(k == 0),
                    stop=(k == KC - 1),
                )
            logit = spool.tile([SPC, C1], F32, tag="logit", name="logit")
            nc.vector.tensor_tensor(
                logit[:], pl[:SPC, :], f2_sb[:], op=mybir.AluOpType.add
            )

            # conv m2/m3 (overlaps the top-k on DVE)
            conv_mgroup(2)
            conv_mgroup(3)

            # ---- top-32 per sample (descending): 4 rounds of max8 ----
            idx_all = spool.tile([SPC, CS], U32, tag="idx", name="idx")
            for r in range(4):
                mx8 = spool.tile([SPC, 8], F32, tag="mx8")
                nc.vector.max(out=mx8[:], in_=logit[:])
                nc.vector.max_index(
                    out=idx_all[:, r * 8 : (r + 1) * 8], in_max=mx8[:], in_values=logit[:]
                )
                if r < 3:
                    nc.vector.match_replace(
                        out=logit[:], in_to_replace=mx8[:], in_values=logit[:],
                        imm_value=-1e30,
                    )

            # idx -> per-partition [CS, 1] columns via DRAM roundtrip
            idx_dram = dpool.tile([SPC, CS], U32)
            nc.sync.dma_start(idx_dram[:], idx_all[:])
            idx_col = [None] * SPC
            for s in range(SPC):
                idx_col[s] = spool.tile([CS, 1], U32, tag=f"idxc{s}", name=f"idxc{s}")
                nc.sync.dma_start(
                    idx_col[s][:],
                    idx_dram[s].rearrange("(p one) -> p one", one=1),
                )

            # ---- gather W2 rows + bias for the selected channels ----
            wsel = [None] * SPC
            b2sel = [None] * SPC
            for s in range(SPC):
                wsel[s] = spool.tile([CS, C1], F32, tag=f"wsel{s}", name=f"wsel{s}")
                nc.gpsimd.indirect_dma_start(
                    out=wsel[s][:],
                    out_offset=None,
                    in_=w2rows.ap()[0:C1, :],
                    in_offset=bass.IndirectOffsetOnAxis(ap=idx_col[s][:, :1], axis=0),
                )
                b2sel[s] = spool.tile([CS, 1], F32, tag=f"b2s{s}", name=f"b2s{s}")
                nc.gpsimd.indirect_dma_start(
                    out=b2sel[s][:],
                    out_offset=None,
                    in_=b2vec.ap().rearrange("(c one) -> c one", one=1),
                    in_offset=bass.IndirectOffsetOnAxis(ap=idx_col[s][:, :1], axis=0),
                )

            # ---- selconv: xsel = W2[idx] @ x + b2[idx]  (per sample) ----
            xsel = [None] * SPC
            for s in range(SPC):
                # transpose gathered rows -> lhsT tiles [c, 32]
                ptr = psB.tile([P, NF], F32, tag="pa", name=f"ptr{s}")
                for k in range(KC):
                    nc.tensor.transpose(
                        ptr[:, k * CS : (k + 1) * CS],
                        wsel[s][:, k * P : (k + 1) * P],
                        i32_sb[:],
                    )
                wselT = spool.tile([P, KC, CS], F32R, tag=f"wT{s}", name=f"wselT{s}")
                nc.scalar.activation(
                    wselT[:], ptr[:, 0 : KC * CS], mybir.ActivationFunctionType.Copy
                )
                xsel[s] = spool.tile([CS, HWD], F32R, tag=f"xsel{s}", name=f"xsel{s}")
                for n in range(NNC):
                    psel = psA.tile([P, NF], F32, tag="cv", name=f"psel{s}_{n}")
                    for k in range(KC):
                        nc.tensor.matmul(
                            psel[:CS, :],
                            lhsT=wselT[:, k, :],
                            rhs=x_sb[s][:, k, n * NF : (n + 1) * NF],
                            start=(k == 0),
                            stop=(k == KC - 1),
                        )
                    nc.scalar.activation(
                        xsel[s][:, n * NF : (n + 1) * NF],
                        psel[:CS, :],
                        mybir.ActivationFunctionType.Identity,
                        bias=b2sel[s][:, 0:1],
                        scale=1.0,
                    )

            # ---- hadamard expansion + batch stats ----
            # per m-chunk: pa = Ghi@xsel (4x), pb = Ghj@xsel (4x); Pool copies
            # pa->SBUF; DVE ttr: prod = a*pb with s1 accumulation chained into
            # stats[:, m]; ACT Square+accum -> s2p slots.
            for m, (mo, mw) in enumerate(EXP_M):
                pas = {}
                for s in range(SPC):
                    for n in range(NNC):
                        pas[(s, n)] = psB.tile([P, NF], F32, tag="pa", name=f"pa{m}_{s}_{n}")
                        nc.tensor.matmul(
                            pas[(s, n)][:mw, :],
                            lhsT=ghi_sb[:, mo : mo + mw],
                            rhs=xsel[s][:, n * NF : (n + 1) * NF],
                            start=True,
                            stop=True,
                        )
                pbs = {}
                for s in range(SPC):
                    for n in range(NNC):
                        pbs[(s, n)] = psA.tile([P, NF], F32, tag="cv", name=f"pb{m}_{s}_{n}")
                        nc.tensor.matmul(
                            pbs[(s, n)][:mw, :],
                            lhsT=ghj_sb[:, mo : mo + mw],
                            rhs=xsel[s][:, n * NF : (n + 1) * NF],
                            start=True,
                            stop=True,
                        )
                for s in range(SPC):
                    for n in range(NNC):
                        a_sb = apool.tile([P, NF], F32, tag="ac")
                        nc.scalar.activation(
                            a_sb[:mw, :], pas[(s, n)][:mw, :],
                            mybir.ActivationFunctionType.Copy,
                        )
                        pchunk = prod_sb[s][:, m, n * NF : (n + 1) * NF]
                        nc.vector.tensor_tensor(
                            pchunk[:mw], a_sb[:mw, :], pbs[(s, n)][:mw, :],
                            op=mybir.AluOpType.mult,
                        )
                        nc.vector.tensor_reduce(
                            s1p[:mw, m, n * SPC + s : n * SPC + s + 1],
                            pchunk[:mw],
                            axis=mybir.AxisListType.X,
                            op=mybir.AluOpType.add,
                        )
                        sq = apool.tile([P, NF], F32, tag="sq")
                        nc.scalar.activation(
                            sq[:mw, :],
                            pchunk[:mw],
                            mybir.ActivationFunctionType.Square,
                            accum_out=s2p[:mw, m, n * SPC + s : n * SPC + s + 1],
                        )

            # s1/s2 slots -> stats cols
            nc.vector.tensor_reduce(
                stats[:, 0:MC], s1p[:], axis=mybir.AxisListType.X,
                op=mybir.AluOpType.add,
            )
            nc.vector.tensor_reduce(
                stats[:, MC : 2 * MC], s2p[:], axis=mybir.AxisListType.X,
                op=mybir.AluOpType.add,
            )

            # ---- cross-core AllReduce of [s1|s2] ----
            cc_in = dpool.tile([P, 2 * MC], F32)
            cc_out = dpool.tile([P, 2 * MC], F32)
            nc.sync.dma_start(cc_in[:], stats[:])
            nc.gpsimd.collective_compute(
                "AllReduce",
                mybir.AluOpType.add,
                replica_groups=[list(range(NCORES))],
                ins=[cc_in.opt()],
                outs=[cc_out.opt()],
            )
            nc.sync.dma_start(gstats[:], cc_out[:])

            # mean/var -> alpha = rstd*gpair ; beta2 = bpair - mean*alpha
            meanc = spool.tile([P, MC], F32, tag="meanc", name="meanc")
            nc.vector.tensor_scalar_mul(meanc[:], gstats[:, 0:MC], 1.0 / NTOT)
            varc = spool.tile([P, MC], F32, tag="varc", name="varc")
            nc.vector.tensor_scalar_mul(varc[:], gstats[:, MC : 2 * MC], 1.0 / NTOT)
            msq = spool.tile([P, MC], F32, tag="msq", name="msq")
            nc.vector.tensor_mul(msq[:], meanc[:], meanc[:])
            nc.vector.tensor_sub(varc[:], varc[:], msq[:])
            nc.scalar.activation(
                varc[:], varc[:], mybir.ActivationFunctionType.Sqrt,
                bias=eps_col[:, 0:1],
            )
            rstd = spool.tile([P, MC], F32, tag="rstd", name="rstd")
            nc.vector.reciprocal(rstd[:], varc[:])
            alpha = spool.tile([P, MC], F32, tag="alpha", name="alpha")
            nc.vector.tensor_mul(alpha[:], rstd[:], gpair_v[:])
            beta2 = spool.tile([P, MC], F32, tag="beta2", name="beta2")
            nc.vector.tensor_mul(beta2[:], meanc[:], alpha[:])
            nc.vector.tensor_sub(beta2[:], bpair_v[:], beta2[:])

            # ---- normalize + write prod rows (streamed) ----
            for s in range(SPC):
                for m, (mo, mw) in enumerate(EXP_M):
                    for n in range(NNC):
                        pchunk = prod_sb[s][:, m, n * NF : (n + 1) * NF]
                        nc.vector.tensor_scalar(
                            pchunk[:mw],
                            pchunk[:mw],
                            alpha[:mw, m : m + 1],
                            beta2[:mw, m : m + 1],
                            op0=mybir.AluOpType.mult,
                            op1=mybir.AluOpType.add,
                        )
                        nc.sync.dma_start(
                            outs[s].ap()[
                                C1 + mo : C1 + mo + mw, n * NF : (n + 1) * NF
                            ],
                            pchunk[:mw],
                        )

    nc.compile()
    return nc


_NC_CACHE = {}


def _get_program():
    if "nc" not in _NC_CACHE:
        _NC_CACHE["nc"] = build_program()
    return _NC_CACHE["nc"]


def _make_consts():
    ghi = np.zeros((CS, CSE), np.float32)
    ghj = np.zeros((CS, CSE), np.float32)
    ghi[HI, np.arange(CSE)] = 1.0
    ghj[HJ, np.arange(CSE)] = 1.0
    return ghi, ghj


def make_shared_inputs(fc_w, fc_b, bn_gamma, bn_beta, bn_mean, bn_var, eva_w, eva_b):
    g64 = np.asarray(bn_gamma, np.float64)
    s64 = g64 / np.sqrt(np.asarray(bn_var, np.float64) + EPS)
    W2_64 = s64[:, None] * np.asarray(fc_w, np.float64)
    b2_64 = (np.asarray(fc_b, np.float64) - np.asarray(bn_mean, np.float64)) * s64 \
        + np.asarray(bn_beta, np.float64)
    E64 = (np.asarray(eva_w, np.float64) @ W2_64) / float(HWD)
    f64 = np.asarray(eva_w, np.float64) @ b2_64 + np.asarray(eva_b, np.float64)

    W2 = W2_64.astype(np.float32)
    gam = np.asarray(bn_gamma, np.float32)
    bet = np.asarray(bn_beta, np.float32)
    gpair = np.zeros(C1, np.float32)
    bpair = np.zeros(C1, np.float32)
    gpair[:CSE] = gam[HI] * gam[HJ]
    bpair[:CSE] = bet[HI] * bet[HJ]
    ghi, ghj = _make_consts()
    return dict(
        w2T=np.ascontiguousarray(W2.T),
        w2rows=np.ascontiguousarray(W2),
        eT=np.ascontiguousarray(E64.astype(np.float32).T),
        f2=np.broadcast_to(f64.astype(np.float32), (SPC, C1)).copy(),
        b2vec=b2_64.astype(np.float32),
        gpairp=gpair,
        bpairp=bpair,
        g_hi=ghi,
        g_hj=ghj,
        ident32=np.eye(CS, dtype=np.float32),
    )


def make_in_maps(inputs):
    x = np.asarray(inputs["x"], np.float32).reshape(B, C1, HWD)
    shared = make_shared_inputs(
        inputs["fc_w"], inputs["fc_b"], inputs["bn_gamma"], inputs["bn_beta"],
        inputs["bn_mean"], inputs["bn_var"], inputs["eva_w"], inputs["eva_b"],
    )
    return [
        dict(shared, xs=np.ascontiguousarray(x[i * SPC : (i + 1) * SPC]))
        for i in range(NCORES)
    ]


def kernel(x, fc_w, fc_b, bn_gamma, bn_beta, bn_mean, bn_var, eva_w, eva_b):
    in_maps = make_in_maps(dict(
        x=x, fc_w=fc_w, fc_b=fc_b, bn_gamma=bn_gamma, bn_beta=bn_beta,
        bn_mean=bn_mean, bn_var=bn_var, eva_w=eva_w, eva_b=eva_b,
    ))
    nc = _get_program()
    res = run_bass_kernel_spmd(nc, in_maps, list(range(NCORES))).results
    out = np.empty((B, C1 + CSE, HWD), np.float32)
    for i in range(NCORES):
        for s in range(SPC):
            out[i * SPC + s] = res[i][f"out{s}"]
    return out.reshape(B, C1 + CSE, 32, 32)
